# revision 52
# baseline (speedup 1.0000x reference)
"""LongcatMoe Trainium2 kernel — expert-parallel sparse MoE across 8 NeuronCores.

Strategy (expert-parallel, per the sharding hint):
  - Host computes the tiny router (fp64 softmax/top-k) and dispatches tokens
    by top-k expert id: core e receives the tokens routed to expert e (padded
    to capacity C=256, capacity factor 1.0; overflow falls back to an exact
    host computation), plus expert e's weights.
  - Each core runs the silu-gated MLP for its expert on its token block.
  - Host combines: out[tok] += gate_weight * y, plus the zero-expert
    (identity) term zero_w[t] * x[t].

Device kernel: fp8 DoubleRow everywhere, no transposes, one DMA stream.
  Both phases contract over the partition dim with DoubleRow (K=256 per
  step, 2 fp8 rows/cycle):
    Phase 1 (gate/up): stationary w{g,u} blocks [128, 2, 128], moving
      x8 [128, 2, C] -> mid[j*128+p, c] accumulates in PSUM [128, C] —
      mid lands directly in [I, C] layout, no transpose needed.
    Phase 2 (down):   stationary w_down blocks [128, 2, 128], moving
      mid8 [128, 2, C] -> y[k] PSUM [128, C].
  All quantization scales are exact powers of two:
      x *= 2^4, w_gate/w_up *= 2^7          (fp8 e4m3, normal range)
      silu via activation(scale=2^-11) recovers the true gate
      mid8 = (silu * u_psum) * 2^-7          (= mid * 2^4, fp8)
      w_down *= 2^7; y = psum * 2^-11        (copy-with-scale on ACT)
  Rel-err vs reference (fixed seed, host-simulated == HW measured):
  ~1.92e-2, under the 2e-2 gate.

DMA design (v2-v5 trace analysis): DMA transfers serialize in global
semaphore-rotation order, which follows the scheduler's simulated start
times — any late-issued transfer (e.g. a mid transpose) that lands between
weight transfers in that rotation blocks them. v6 therefore has NO
transposes and exactly ONE stream on the SP ring in consumption order:
x8, 8x512KB wgu chunks (j-major), 2x1MB wd chunks, 4x256KB y writes.
The ACT ring carries no DMAs at all.

Host-side layouts (per-partition contiguous for every device DMA):
  x8   [128, KH, 2, C] fp8   x8[p,kh,o,c]      = x[idx[c], kh*256+o*128+p]*2^4
  wgu8 [IO, 128, KH, 2, 2, 128] fp8
                             wgu8[j,p,kh,g,o,m] = w{g}[kh*256+o*128+p,
                                                       j*128+m]*2^7
  wd8  [2, 128, 8, 4, 2, 128] fp8
                             wd8[s,p,k,ip,o,c] = w_down[ip*256+o*128+p,
                                                        (8s+k)*128+c]*2^7
  y    [8, 128, 2, C] bf16 output; host reassembles [H, C]
"""

import os

import numpy as np
import ml_dtypes

T, H, I, E, Z, TOPK = 1024, 2048, 1024, 8, 8, 4
ROUTED_SCALING = 1.0
N_CORES = 8
P = 128
HO = H // P   # 16
IO = I // P   # 8
KH = H // 256  # 8 DoubleRow k-groups (phase 1)
IP = I // 256  # 4 DoubleRow k-groups (phase 2)
C = 256       # per-expert token capacity (capacity factor 1.0; overflow → host)
NWARM = 12    # coarse warmup matmuls (N=256, ~256ns each)
NWARM2 = 26   # fine warmup matmuls (N=128) — end near worst-case data-ready

XS = 2.0 ** 4    # x fp8 pre-scale
WS = 2.0 ** 7    # w_gate/w_up/w_down fp8 pre-scale
SSIL = 2.0 ** -11  # silu input scale (1/(XS*WS))
SMID = 2.0 ** -7   # mid8 = silu*u_psum*SMID = mid*2^4
SY = 2.0 ** -11    # y = psum/(2^4 * 2^7)

_PROGRAM = None
LAST_RESULTS = None  # BassKernelResults of the most recent run (for test harness)


def _build_program():
    import concourse.mybir as mybir
    import concourse.tile as tile
    from concourse import bacc

    f32 = mybir.dt.float32
    bf16 = mybir.dt.bfloat16
    fp8 = mybir.dt.float8e4
    SILU = mybir.ActivationFunctionType.Silu
    COPY = mybir.ActivationFunctionType.Copy
    DR = mybir.MatmulPerfMode.DoubleRow

    nc = bacc.Bacc(
        "TRN2",
        target_bir_lowering=False,
        debug=False,
        enable_asserts=False,
        num_devices=N_CORES,
    )
    x8 = nc.dram_tensor("x8", [P, KH, 2, C], fp8, kind="ExternalInput").ap()
    wgu8 = nc.dram_tensor("wgu8", [IO, P, KH, 2, 2, P], fp8,
                          kind="ExternalInput").ap()
    wd8 = nc.dram_tensor("wd8", [2, P, HO // 2, IP, 2, P], fp8,
                         kind="ExternalInput").ap()
    y = nc.dram_tensor("y", [8, P, 2, C], bf16, kind="ExternalOutput").ap()

    with tile.TileContext(nc) as tc:
        with (
            tc.tile_pool(name="px", bufs=1) as px,
            tc.tile_pool(name="pwgu", bufs=IO) as pwgu,
            tc.tile_pool(name="pwd", bufs=2) as pwd,
            tc.tile_pool(name="pmidf", bufs=2) as pmidf,
            tc.tile_pool(name="pmid8", bufs=1) as pmid8,
            tc.tile_pool(name="psg", bufs=2) as psg,
            tc.tile_pool(name="py", bufs=4) as py,
            tc.tile_pool(name="pwrm", bufs=1) as pwrm,
            tc.tile_pool(name="ppg", bufs=2, space="PSUM") as ppg,
            tc.tile_pool(name="ppu", bufs=2, space="PSUM") as ppu,
            tc.tile_pool(name="ppd", bufs=3, space="PSUM") as ppd,
            tc.tile_pool(name="ppw", bufs=1, space="PSUM") as ppw,
        ):
            # PE warmup: keep the tensor engine busy while the head DMAs land
            # so the HAM clock-gate reaches 2.4 GHz by the first real matmul.
            wtile = pwrm.tile([P, C], bf16)
            nc.gpsimd.memset(wtile[:], 0.0)
            pwm = ppw.tile([P, C], f32)
            for w in range(NWARM):
                nc.tensor.matmul(pwm[:], wtile[:, :P], wtile[:],
                                 start=(w == 0), stop=(w == NWARM - 1))
            for w in range(NWARM2):
                nc.tensor.matmul(pwm[:, :P], wtile[:, :P], wtile[:, :P],
                                 start=(w == 0), stop=(w == NWARM2 - 1))

            xt = px.tile([P, KH, 2, C], fp8)
            wgu_t = [pwgu.tile([P, KH, 2, 2, P], fp8, name=f"wgu{j}",
                               tag="wgu") for j in range(IO)]
            wd_t = [pwd.tile([P, HO // 2, IP, 2, P], fp8, name=f"wd{s}",
                             tag="wd") for s in range(2)]
            mid8 = pmid8.tile([P, IO, C], fp8)

            # ONE DMA stream on the SP ring in exact consumption order.
            # The head is split (x8 kh-halves, wgu0 by gate/up; all chunks
            # keep >=2KB partition lines — sub-1KB lines are descriptor-bound
            # while the DMA path ramps) so the first real matmul only needs
            # 512KB instead of 1MB off the slow early stream.
            with tc.high_priority():
                hk = KH // 2
                nc.sync.dma_start(xt[:, 0:hk], x8[:, 0:hk])
                nc.sync.dma_start(wgu_t[0][:, 0:hk], wgu8[0][:, 0:hk])
                nc.sync.dma_start(xt[:, hk:KH], x8[:, hk:KH])
                nc.sync.dma_start(wgu_t[0][:, hk:KH], wgu8[0][:, hk:KH])
                for j in range(1, IO):
                    nc.sync.dma_start(wgu_t[j][:], wgu8[j])
                for s in range(2):
                    nc.sync.dma_start(wd_t[s][:], wd8[s])

            # Phase 1: mid[j] = silu(x @ Wg_j) * (x @ Wu_j) in [I, C] layout,
            # fp8 DoubleRow with stationary weight blocks.
            for j in range(IO):
                pg = ppg.tile([P, C], f32)
                pu = ppu.tile([P, C], f32)
                # Last j: all gate matmuls first so the silu overlaps the up
                # accumulation — shortens the phase-1 -> phase-2 drain chain.
                # Other j's interleave gate/up per kh to follow the kh-major
                # DMA chunks with no stalls.
                if j == IO - 1:
                    order = [(kh, g) for g in range(2) for kh in range(KH)]
                else:
                    order = [(kh, g) for kh in range(KH) for g in range(2)]
                for kh, g in order:
                    nc.tensor.matmul(
                        pg[:] if g == 0 else pu[:],
                        wgu_t[j][:, kh, g, :, :], xt[:, kh, :, :],
                        start=(kh == 0), stop=(kh == KH - 1),
                        perf_mode=DR,
                    )
                sg = psg.tile([P, C], f32)
                nc.scalar.activation(sg[:], pg[:], SILU, scale=SSIL)
                midf = pmidf.tile([P, C], f32)
                nc.vector.tensor_mul(out=midf[:], in0=sg[:], in1=pu[:])
                nc.scalar.activation(mid8[:, j, :], midf[:], COPY, scale=SMID)

            # Phase 2: y[k] = mid @ Wd[k], fp8 DoubleRow, [H, C] layout;
            # y tiles written out in pairs on the SP ring.
            for kq in range(8):
                ty = py.tile([P, 2, C], bf16)
                for sub in range(2):
                    k = kq * 2 + sub
                    pd = ppd.tile([P, C], f32)
                    for ip in range(IP):
                        nc.tensor.matmul(
                            pd[:],
                            wd_t[k // (HO // 2)][:, k % (HO // 2), ip, :, :],
                            mid8[:, 2 * ip:2 * ip + 2, :],
                            start=(ip == 0), stop=(ip == IP - 1),
                            perf_mode=DR,
                        )
                    nc.scalar.activation(ty[:, sub, :], pd[:], COPY, scale=SY)
                    if kq == 7:
                        # Split the final pair so the last transfer is small.
                        nc.sync.dma_start(y[kq][:, sub:sub + 1, :],
                                          ty[:, sub:sub + 1, :])
                if kq < 7:
                    nc.sync.dma_start(y[kq], ty[:])

    nc.compile()
    return nc


def _route(x, router_w, corr_bias):
    """fp64 router: returns (topk_idx [T,K], topk_w [T,K])."""
    xl = x.astype(np.float64)
    logits = xl @ router_w.astype(np.float64).T
    logits -= logits.max(axis=1, keepdims=True)
    p = np.exp(logits)
    p /= p.sum(axis=1, keepdims=True)
    sel = p + corr_bias.astype(np.float64)
    topk_idx = np.argsort(-sel, axis=1, kind="stable")[:, :TOPK]
    topk_w = np.take_along_axis(p, topk_idx, axis=1) * ROUTED_SCALING
    return topk_idx, topk_w


def kernel(hidden_states, router_w, corr_bias, w_gate, w_up, w_down):
    global _PROGRAM, LAST_RESULTS
    x = np.asarray(hidden_states, dtype=np.float32)
    router_w = np.asarray(router_w, dtype=np.float32)
    corr_bias = np.asarray(corr_bias, dtype=np.float32)
    w_gate = np.asarray(w_gate, dtype=np.float32)
    w_up = np.asarray(w_up, dtype=np.float32)
    w_down = np.asarray(w_down, dtype=np.float32)

    topk_idx, topk_w = _route(x, router_w, corr_bias)
    routed = topk_idx < E
    zero_w = (topk_w * (~routed)).sum(axis=1)  # [T] fp64

    e4 = ml_dtypes.float8_e4m3

    # Dispatch: token list + gate weight per expert; overflow beyond C
    # falls back to an exact host computation.
    idx_list, w_list, overflow = [], [], []
    for e in range(E):
        toks, kpos = np.nonzero(topk_idx == e)
        we = topk_w[toks, kpos]
        if len(toks) > C:
            overflow.append((e, toks[C:], we[C:]))
            toks, we = toks[:C], we[:C]
        idx_list.append(toks)
        w_list.append(we)

    in_maps = []
    for e in range(E):
        toks = idx_list[e]
        n = len(toks)
        xg = np.zeros((C, H), dtype=np.float32)
        xg[:n] = x[toks]
        # x8[p, kh, o, c] = x[c, kh*256+o*128+p] * XS
        x8d = np.ascontiguousarray(
            (xg * XS).astype(e4).reshape(C, KH, 2, P).transpose(3, 1, 2, 0))
        # wgu8[j, p, kh, g, o, m] = w{g}[kh*256+o*128+p, j*128+m] * WS
        wg8 = (w_gate[e] * WS).astype(e4)
        wu8 = (w_up[e] * WS).astype(e4)
        # [g, H, I] -> [g, kh, o, p, j, m] -> [j, p, kh, g, o, m]
        wgu_s = np.stack([wg8, wu8], axis=0).reshape(2, KH, 2, P, IO, P)
        wgud = np.ascontiguousarray(wgu_s.transpose(4, 3, 1, 0, 2, 5))
        # wd8[s, p, k, ip, o, c] = w_down[ip*256+o*128+p, (8s+k)*128+c] * WS
        # [I, H] -> [ip, o, p, s, k, c] -> [s, p, k, ip, o, c]
        wd_s = (w_down[e] * WS).astype(e4).reshape(IP, 2, P, 2, HO // 2, P)
        wdd = np.ascontiguousarray(wd_s.transpose(3, 2, 4, 0, 1, 5))
        in_maps.append({"x8": x8d, "wgu8": wgud, "wd8": wdd})

    if _PROGRAM is None:
        _PROGRAM = _build_program()

    from concourse.bass_utils import run_bass_kernel_spmd

    kw = {}
    if os.environ.get("MOE_KERNEL_TRACE", "") == "1":
        kw = dict(trace=True, trace_cores=list(range(N_CORES)))
    res = run_bass_kernel_spmd(
        _PROGRAM, in_maps, core_ids=list(range(N_CORES)), **kw)
    LAST_RESULTS = res

    out = np.zeros((T, H), dtype=np.float64)
    for e in range(E):
        n = len(idx_list[e])
        if n:
            yr = res.results[e]["y"]  # [8, P, 2, C] bf16
            ye = yr.transpose(0, 2, 1, 3).reshape(H, C)
            out[idx_list[e]] += (w_list[e][:, None]
                                 * ye[:, :n].T.astype(np.float64))
    for e, toks, ws in overflow:
        xt = x[toks].astype(np.float64)
        g = xt @ w_gate[e].astype(np.float64)
        u = xt @ w_up[e].astype(np.float64)
        mid = (g / (1.0 + np.exp(-g))) * u
        out[toks] += ws[:, None] * (mid @ w_down[e].astype(np.float64))
    out += zero_w[:, None] * x.astype(np.float64)
    return out.astype(np.float32)


# revision 53
# speedup vs baseline: 1.1453x; 1.1453x over previous
"""LongcatMoe Trainium2 kernel — expert-parallel sparse MoE across 8 NeuronCores.

Strategy (expert-parallel, per the sharding hint):
  - Host computes the tiny router (fp64 softmax/top-k) and dispatches tokens
    by top-k expert id: core e receives the tokens routed to expert e (padded
    to capacity C=256, capacity factor 1.0; overflow falls back to an exact
    host computation), plus expert e's weights.
  - Each core runs the silu-gated MLP for its expert on its token block.
  - Host combines: out[tok] += gate_weight * y, plus the zero-expert
    (identity) term zero_w[t] * x[t].

Device kernel: fp8 DoubleRow everywhere, no transposes, one DMA stream.
  Both phases contract over the partition dim with DoubleRow (K=256 per
  step, 2 fp8 rows/cycle):
    Phase 1 (gate/up): stationary w{g,u} blocks [128, 2, 128], moving
      x8 [128, 2, C] -> mid[j*128+p, c] accumulates in PSUM [128, C] —
      mid lands directly in [I, C] layout, no transpose needed.
    Phase 2 (down):   stationary w_down blocks [128, 2, 128], moving
      mid8 [128, 2, C] -> y[k] PSUM [128, C].
  All quantization scales are exact powers of two:
      x *= 2^4, w_gate/w_up *= 2^7          (fp8 e4m3, normal range)
      silu via activation(scale=2^-11) recovers the true gate
      mid8 = (silu * u_psum) * 2^-7          (= mid * 2^4, fp8)
      w_down *= 2^7; y = psum * 2^-11        (copy-with-scale on ACT)
  Rel-err vs reference (fixed seed, host-simulated == HW measured):
  ~1.92e-2, under the 2e-2 gate.

DMA design (v2-v5 trace analysis): DMA transfers serialize in global
semaphore-rotation order, which follows the scheduler's simulated start
times — any late-issued transfer (e.g. a mid transpose) that lands between
weight transfers in that rotation blocks them. v6 therefore has NO
transposes and exactly ONE stream on the SP ring in consumption order:
x8, 8x512KB wgu chunks (j-major), 2x1MB wd chunks, 4x256KB y writes.
The ACT ring carries no DMAs at all.

Host-side layouts (per-partition contiguous for every device DMA):
  x8   [128, KH, 2, C] fp8   x8[p,kh,o,c]      = x[idx[c], kh*256+o*128+p]*2^4
  wgu8 [IO, 128, KH, 2, 2, 128] fp8
                             wgu8[j,p,kh,g,o,m] = w{g}[kh*256+o*128+p,
                                                       j*128+m]*2^7
  wd8  [2, 128, 8, 4, 2, 128] fp8
                             wd8[s,p,k,ip,o,c] = w_down[ip*256+o*128+p,
                                                        (8s+k)*128+c]*2^7
  y    [8, 128, 2, C] bf16 output; host reassembles [H, C]
"""

import os

import numpy as np
import ml_dtypes

T, H, I, E, Z, TOPK = 1024, 2048, 1024, 8, 8, 4
ROUTED_SCALING = 1.0
N_CORES = 8
P = 128
HO = H // P   # 16
IO = I // P   # 8
KH = H // 256  # 8 DoubleRow k-groups (phase 1)
IP = I // 256  # 4 DoubleRow k-groups (phase 2)
C = 256       # per-expert token capacity (capacity factor 1.0; overflow → host)
NWARM = 12    # coarse warmup matmuls (N=256, ~256ns each)
NWARM2 = 26   # fine warmup matmuls (N=128) — end near worst-case data-ready

XS = 2.0 ** 4    # x fp8 pre-scale
WS = 2.0 ** 7    # w_gate/w_up/w_down fp8 pre-scale
SSIL = 2.0 ** -11  # silu input scale (1/(XS*WS))
SMID = 2.0 ** -7   # mid8 = silu*u_psum*SMID = mid*2^4
SY = 2.0 ** -11    # y = psum/(2^4 * 2^7)

_PROGRAM = None
LAST_RESULTS = None  # BassKernelResults of the most recent run (for test harness)


def _build_program():
    import concourse.mybir as mybir
    import concourse.tile as tile
    from concourse import bacc

    f32 = mybir.dt.float32
    bf16 = mybir.dt.bfloat16
    fp8 = mybir.dt.float8e4
    SILU = mybir.ActivationFunctionType.Silu
    COPY = mybir.ActivationFunctionType.Copy
    DR = mybir.MatmulPerfMode.DoubleRow

    nc = bacc.Bacc(
        "TRN2",
        target_bir_lowering=False,
        debug=False,
        enable_asserts=False,
        num_devices=N_CORES,
    )
    x8 = nc.dram_tensor("x8", [P, KH, 2, C], fp8, kind="ExternalInput").ap()
    wgu8 = nc.dram_tensor("wgu8", [IO, P, KH, 2, 2, P], fp8,
                          kind="ExternalInput").ap()
    wd8 = nc.dram_tensor("wd8", [2, P, HO // 2, IP, 2, P], fp8,
                         kind="ExternalInput").ap()
    y = nc.dram_tensor("y", [8, P, 2, C], bf16, kind="ExternalOutput").ap()

    with tile.TileContext(nc) as tc:
        with (
            tc.tile_pool(name="px", bufs=1) as px,
            tc.tile_pool(name="pwgu", bufs=IO) as pwgu,
            tc.tile_pool(name="pwd", bufs=2) as pwd,
            tc.tile_pool(name="pmidf", bufs=2) as pmidf,
            tc.tile_pool(name="pmid8", bufs=1) as pmid8,
            tc.tile_pool(name="psg", bufs=2) as psg,
            tc.tile_pool(name="py", bufs=4) as py,
            tc.tile_pool(name="pwrm", bufs=1) as pwrm,
            tc.tile_pool(name="ppg", bufs=2, space="PSUM") as ppg,
            tc.tile_pool(name="ppu", bufs=2, space="PSUM") as ppu,
            tc.tile_pool(name="ppd", bufs=3, space="PSUM") as ppd,
            tc.tile_pool(name="ppw", bufs=1, space="PSUM") as ppw,
        ):
            # PE warmup: keep the tensor engine busy while the head DMAs land
            # so the HAM clock-gate reaches 2.4 GHz by the first real matmul.
            wtile = pwrm.tile([P, C], bf16)
            nc.gpsimd.memset(wtile[:], 0.0)
            pwm = ppw.tile([P, C], f32)
            for w in range(NWARM):
                nc.tensor.matmul(pwm[:], wtile[:, :P], wtile[:],
                                 start=(w == 0), stop=(w == NWARM - 1))
            for w in range(NWARM2):
                nc.tensor.matmul(pwm[:, :P], wtile[:, :P], wtile[:, :P],
                                 start=(w == 0), stop=(w == NWARM2 - 1))

            xt = px.tile([P, KH, 2, C], fp8)
            wgu_t = [pwgu.tile([P, KH, 2, 2, P], fp8, name=f"wgu{j}",
                               tag="wgu") for j in range(IO)]
            wd_t = [pwd.tile([P, HO // 2, IP, 2, P], fp8, name=f"wd{s}",
                             tag="wd") for s in range(2)]
            mid8 = pmid8.tile([P, IO, C], fp8)

            # ONE DMA stream on the SP ring in exact consumption order.
            # The head is split (x8 kh-halves, wgu0 by gate/up; all chunks
            # keep >=2KB partition lines — sub-1KB lines are descriptor-bound
            # while the DMA path ramps) so the first real matmul only needs
            # 512KB instead of 1MB off the slow early stream.
            with tc.high_priority():
                nc.sync.dma_start(xt[:], x8[:])
                nc.sync.dma_start(wgu_t[0][:, 0:KH // 2], wgu8[0][:, 0:KH // 2])
                nc.sync.dma_start(wgu_t[0][:, KH // 2:KH],
                                  wgu8[0][:, KH // 2:KH])
                for j in range(1, IO):
                    nc.sync.dma_start(wgu_t[j][:], wgu8[j])
                for s in range(2):
                    nc.sync.dma_start(wd_t[s][:], wd8[s])

            # Phase 1: mid[j] = silu(x @ Wg_j) * (x @ Wu_j) in [I, C] layout,
            # fp8 DoubleRow with stationary weight blocks.
            for j in range(IO):
                pg = ppg.tile([P, C], f32)
                pu = ppu.tile([P, C], f32)
                # Last j: all gate matmuls first so the silu overlaps the up
                # accumulation — shortens the phase-1 -> phase-2 drain chain.
                # Other j's interleave gate/up per kh to follow the kh-major
                # DMA chunks with no stalls.
                if j == IO - 1:
                    order = [(kh, g) for g in range(2) for kh in range(KH)]
                else:
                    order = [(kh, g) for kh in range(KH) for g in range(2)]
                for kh, g in order:
                    nc.tensor.matmul(
                        pg[:] if g == 0 else pu[:],
                        wgu_t[j][:, kh, g, :, :], xt[:, kh, :, :],
                        start=(kh == 0), stop=(kh == KH - 1),
                        perf_mode=DR,
                    )
                sg = psg.tile([P, C], f32)
                nc.scalar.activation(sg[:], pg[:], SILU, scale=SSIL)
                midf = pmidf.tile([P, C], f32)
                nc.vector.tensor_mul(out=midf[:], in0=sg[:], in1=pu[:])
                nc.scalar.activation(mid8[:, j, :], midf[:], COPY, scale=SMID)

            # Phase 2: y[k] = mid @ Wd[k], fp8 DoubleRow, [H, C] layout;
            # y tiles written out in pairs on the SP ring.
            for kq in range(8):
                ty = py.tile([P, 2, C], bf16)
                for sub in range(2):
                    k = kq * 2 + sub
                    pd = ppd.tile([P, C], f32)
                    for ip in range(IP):
                        nc.tensor.matmul(
                            pd[:],
                            wd_t[k // (HO // 2)][:, k % (HO // 2), ip, :, :],
                            mid8[:, 2 * ip:2 * ip + 2, :],
                            start=(ip == 0), stop=(ip == IP - 1),
                            perf_mode=DR,
                        )
                    nc.scalar.activation(ty[:, sub, :], pd[:], COPY, scale=SY)
                    if kq == 7:
                        # Split the final pair so the last transfer is small.
                        nc.sync.dma_start(y[kq][:, sub:sub + 1, :],
                                          ty[:, sub:sub + 1, :])
                if kq < 7:
                    nc.sync.dma_start(y[kq], ty[:])

    nc.compile()
    return nc


def _route(x, router_w, corr_bias):
    """fp64 router: returns (topk_idx [T,K], topk_w [T,K])."""
    xl = x.astype(np.float64)
    logits = xl @ router_w.astype(np.float64).T
    logits -= logits.max(axis=1, keepdims=True)
    p = np.exp(logits)
    p /= p.sum(axis=1, keepdims=True)
    sel = p + corr_bias.astype(np.float64)
    topk_idx = np.argsort(-sel, axis=1, kind="stable")[:, :TOPK]
    topk_w = np.take_along_axis(p, topk_idx, axis=1) * ROUTED_SCALING
    return topk_idx, topk_w


def kernel(hidden_states, router_w, corr_bias, w_gate, w_up, w_down):
    global _PROGRAM, LAST_RESULTS
    x = np.asarray(hidden_states, dtype=np.float32)
    router_w = np.asarray(router_w, dtype=np.float32)
    corr_bias = np.asarray(corr_bias, dtype=np.float32)
    w_gate = np.asarray(w_gate, dtype=np.float32)
    w_up = np.asarray(w_up, dtype=np.float32)
    w_down = np.asarray(w_down, dtype=np.float32)

    topk_idx, topk_w = _route(x, router_w, corr_bias)
    routed = topk_idx < E
    zero_w = (topk_w * (~routed)).sum(axis=1)  # [T] fp64

    e4 = ml_dtypes.float8_e4m3

    # Dispatch: token list + gate weight per expert; overflow beyond C
    # falls back to an exact host computation.
    idx_list, w_list, overflow = [], [], []
    for e in range(E):
        toks, kpos = np.nonzero(topk_idx == e)
        we = topk_w[toks, kpos]
        if len(toks) > C:
            overflow.append((e, toks[C:], we[C:]))
            toks, we = toks[:C], we[:C]
        idx_list.append(toks)
        w_list.append(we)

    in_maps = []
    for e in range(E):
        toks = idx_list[e]
        n = len(toks)
        xg = np.zeros((C, H), dtype=np.float32)
        xg[:n] = x[toks]
        # x8[p, kh, o, c] = x[c, kh*256+o*128+p] * XS
        x8d = np.ascontiguousarray(
            (xg * XS).astype(e4).reshape(C, KH, 2, P).transpose(3, 1, 2, 0))
        # wgu8[j, p, kh, g, o, m] = w{g}[kh*256+o*128+p, j*128+m] * WS
        wg8 = (w_gate[e] * WS).astype(e4)
        wu8 = (w_up[e] * WS).astype(e4)
        # [g, H, I] -> [g, kh, o, p, j, m] -> [j, p, kh, g, o, m]
        wgu_s = np.stack([wg8, wu8], axis=0).reshape(2, KH, 2, P, IO, P)
        wgud = np.ascontiguousarray(wgu_s.transpose(4, 3, 1, 0, 2, 5))
        # wd8[s, p, k, ip, o, c] = w_down[ip*256+o*128+p, (8s+k)*128+c] * WS
        # [I, H] -> [ip, o, p, s, k, c] -> [s, p, k, ip, o, c]
        wd_s = (w_down[e] * WS).astype(e4).reshape(IP, 2, P, 2, HO // 2, P)
        wdd = np.ascontiguousarray(wd_s.transpose(3, 2, 4, 0, 1, 5))
        in_maps.append({"x8": x8d, "wgu8": wgud, "wd8": wdd})

    if _PROGRAM is None:
        _PROGRAM = _build_program()

    from concourse.bass_utils import run_bass_kernel_spmd

    kw = {}
    if os.environ.get("MOE_KERNEL_TRACE", "") == "1":
        kw = dict(trace=True, trace_cores=list(range(N_CORES)))
    res = run_bass_kernel_spmd(
        _PROGRAM, in_maps, core_ids=list(range(N_CORES)), **kw)
    LAST_RESULTS = res

    out = np.zeros((T, H), dtype=np.float64)
    for e in range(E):
        n = len(idx_list[e])
        if n:
            yr = res.results[e]["y"]  # [8, P, 2, C] bf16
            ye = yr.transpose(0, 2, 1, 3).reshape(H, C)
            out[idx_list[e]] += (w_list[e][:, None]
                                 * ye[:, :n].T.astype(np.float64))
    for e, toks, ws in overflow:
        xt = x[toks].astype(np.float64)
        g = xt @ w_gate[e].astype(np.float64)
        u = xt @ w_up[e].astype(np.float64)
        mid = (g / (1.0 + np.exp(-g))) * u
        out[toks] += ws[:, None] * (mid @ w_down[e].astype(np.float64))
    out += zero_w[:, None] * x.astype(np.float64)
    return out.astype(np.float32)
